# Initial kernel scaffold
#
"""Trainium2 Bass kernel for nn_DenseAE (dense autoencoder with top-k masking).

out = sigmoid(relu(ksparse(relu(x @ We1), 64) @ Wd1) @ Wd2)

Data-parallel over 8 NeuronCores: batch 4096 -> 512 rows/core, weights
replicated. Inputs are staged from the host in compute-ready form: x is
pre-transposed and cast to bf16, We1/Wd1 cast to bf16, Wd2 cast to fp8e4
(scaled by 16, descaled inside the sigmoid), so no on-chip casts or input
transposes are needed. Layer 3 runs in fp8 DoubleRow perf mode (2 k-tiles
per matmul). Output is written bf16 and upcast on the host.
"""
import sys

sys.path.insert(0, "/opt/trn_rl_repo")

import numpy as np
import ml_dtypes

import concourse.bass as bass
import concourse.mybir as mybir
import concourse.tile as tile_mod
from concourse.bass_utils import run_bass_kernel_spmd
from concourse.masks import make_identity
from concourse.tile import TileContext
from bass_rust import ScopedClock

F32 = mybir.dt.float32
BF16 = mybir.dt.bfloat16
FP8 = mybir.dt.float8e4
AF = mybir.ActivationFunctionType

P = 128
D = 12288          # 3*64*64
HID = 1024
B = 4096
NCORES = 8
BSH = B // NCORES  # 512 rows per core
NBC = BSH // P     # 4 batch chunks of 128
DKT = D // P       # 96 contraction tiles for layer 1
HKT = HID // P     # 8 contraction tiles for layers 2/3
NCH = D // 512     # 24 output column chunks for layer 3
TOPK = 64
W2SCALE = 16.0     # Wd2 is staged as fp8(Wd2 * W2SCALE)
D_SPLIT = True     # run layer 3 in two bc-pair passes (Wd2 streamed twice) so
                   # the first pass overlaps the top-k of the second pair
TAILKB = 4         # last TAILKB k-blocks of layer 1 run bc-major so h (and the
                   # top-k) of early bc chunks completes before layer 1 ends
W2PRE = 8          # Wd2 nch-chunks prefetched to SBUF during layer 1; the rest
                   # loads as layer-1 SBUF frees. All of Wd2 is read once and
                   # stays resident, so both phase-D passes hit SBUF only.
D_JOUTER = True    # phase D inner order j-outer/bc-inner (interleaves PSUM
                   # accumulation groups, helps PE ld/stream pipelining)


def _patched_drain_and_barrier(self, tick_clock, wait_clock):
    # Tile's kernel-tail drain piles one wait per outstanding proc onto a
    # single Drain instruction; walrus CTRL_NO codegen has fewer wait slots.
    # Keep one wait on the drain and emit the rest as standalone waits.
    drain_inst = self.nc.sync.drain()
    wait_clock.add_sem_waits(drain_inst.ins, ScopedClock({None: tick_clock.global_clock}))
    si = drain_inst.ins.sync_info
    waits = list(si.on_wait or []) if si is not None else []
    if len(waits) > 1:
        si.on_wait = [waits[0]]
        sems = {h.name: h for h in self.sems.allocated().values()}
        for w in waits[1:]:
            assert w.wait_mode == "sem-ge-imm", w
            self.nc.sync.wait_ge(sems[w.ant_name], w.wait_value)
    self.nc.all_engine_barrier()
    assert self.sems is not None
    popped = self.nc._tile_sem_poison_stack.pop()
    assert popped is self._sem_poison
    self.nc.clear_and_free_semaphores(list(self.sems.allocated().values()))
    self.nc.all_engine_barrier()


tile_mod.TileContext._drain_and_barrier = _patched_drain_and_barrier

_WSPLIT_CTR = [0]


def _split_excess_waits(nc, cap=1):
    """This walrus build rejects instructions carrying more than ~1-2 sem
    waits ("Too many sync wait commands"), while Tile freely attaches
    several. Post-pass: keep `cap` waits on each instruction, move the rest
    onto fresh NOPs inserted immediately before it on the same engine."""
    for bb in nc.m.functions[0].blocks:
        lst = bb.instructions
        i = 0
        while i < len(lst):
            inst = lst[i]
            si = inst.sync_info
            waits = list(si.on_wait) if si is not None and si.on_wait else []
            if len(waits) > cap:
                si.on_wait = waits[-cap:]
                for w in waits[:-cap]:
                    nop = mybir.InstNoOp(name=f"wsplit-{_WSPLIT_CTR[0]}")
                    _WSPLIT_CTR[0] += 1
                    nop.engine = inst.engine
                    nop.sync_info = mybir.SyncInfo(on_wait=[w], on_update=[])
                    lst.insert(i, nop)
                    i += 1
            i += 1


def _emit(nc, tc, t, has_bias, rep=1):
    with tc.tile_pool(name="cpool", bufs=1) as const, \
         tc.tile_pool(name="wd1p", bufs=1) as wd1p:
        ident = const.tile([P, P], BF16, name="ident")
        make_identity(nc, ident)
        if has_bias:
            be1_t = const.tile([P, HID], F32, name="be1t")
            nc.sync.dma_start(be1_t, t["be1"][None, :].to_broadcast([P, HID]))
            bd1_t = const.tile([P, HID], F32, name="bd1t")
            nc.sync.dma_start(bd1_t, t["bd1"][None, :].to_broadcast([P, HID]))

        # Wd1 (2 MB bf16): load once up front, on the gpsimd queue so it does
        # not delay the first xt/we1 tiles on the sync queue.
        wd1_sb = wd1p.tile([P, HKT, HID], BF16, name="wd1sb")
        for kk in range(4):
            nc.gpsimd.dma_start(wd1_sb[:, 2 * kk:2 * kk + 2, :],
                                t["wd1"][:, 2 * kk:2 * kk + 2, :])

        for _ in range(rep):
            _emit_main(nc, tc, t, has_bias, ident, wd1_sb,
                       be1_t if has_bias else None, bd1_t if has_bias else None)


def _emit_main(nc, tc, t, has_bias, ident, wd1_sb, be1_t, bd1_t):
    XT, WE1, WD2, bd2, OUT = t["xt"], t["we1"], t["wd2"], t["bd2"], t["out"]

    hs_cm = tc.tile_pool(name="hsp", bufs=1)
    hsp = hs_cm.__enter__()
    h_sb = [hsp.tile([P, HID], BF16, name=f"hsb{bc}") for bc in range(NBC)]

    # Wd2 residency: first W2PRE chunks prefetch during layer 1 (their pool
    # coexists with the layer-1 pools; the DMAs are interleaved into the
    # layer-1 sync-queue stream so they pace evenly instead of bursting at
    # t=0), the rest stream into space recycled from the layer-1 pools.
    # Both phase-D passes then read SBUF only.
    w2c_cm = tc.tile_pool(name="w2c", bufs=1)
    w2cp = w2c_cm.__enter__()
    w2c = w2cp.tile([P, W2PRE, HKT, 512], FP8, name="w2c")

    # Pinned pools for the top-k working set: allocated alongside the
    # layer-1 pools so their first writes are not gated on layer-1 SBUF
    # frees (the DVE top-k must start during the layer-1 tail).
    pin_cms = [tc.tile_pool(name=n, bufs=2) for n in ("scr", "hm", "htl", "dbfp")]
    scr, hm, htl, dbfp = [cm.__enter__() for cm in pin_cms]

    # ---- Phase B: layer 1, h = relu(x @ We1) ----
    # xt block boundaries in k-tiles: two small lead blocks shorten the
    # time-to-first-matmul, the last TAILKB blocks run bc-major so h of
    # early bc chunks (and their DVE top-k) overlaps the tail matmuls.
    bounds = [0, 2, 4, 8] + [8 + 8 * i for i in range(1, (DKT - 8) // 8 + 1)]
    blocks = list(zip(bounds[:-1], bounds[1:]))
    head, tail = blocks[:-TAILKB], blocks[-TAILKB:]

    def l1_mm(h_acc, xt_t, w1, kt, k0, jj, bc):
        first, last = kt == 0, kt == DKT - 1
        lhs = xt_t[:, kt - k0, bc * P:(bc + 1) * P]
        nc.tensor.matmul(h_acc[:, bc * HID:bc * HID + 512], lhs,
                         w1[:, jj, 0:512], start=first, stop=last)
        nc.tensor.matmul(h_acc[:, bc * HID + 512:(bc + 1) * HID], lhs,
                         w1[:, jj, 512:1024], start=first, stop=last)

    def relu_bc(h_acc, bc, htp):
        if has_bias:
            h_tmp = htp.tile([P, HID], F32, name="htmp")
            nc.vector.tensor_add(h_tmp, h_acc[:, bc * HID:(bc + 1) * HID], be1_t)
            nc.scalar.activation(h_sb[bc], h_tmp, AF.Relu)
        else:
            nc.scalar.activation(h_sb[bc], h_acc[:, bc * HID:(bc + 1) * HID], AF.Relu)

    with tc.tile_pool(name="xtp", bufs=max(3, TAILKB)) as xtp, \
         tc.tile_pool(name="w1p", bufs=6) as w1p, \
         tc.tile_pool(name="w1tl", bufs=4 * TAILKB) as w1tl, \
         tc.tile_pool(name="htmp", bufs=2) as htp, \
         tc.tile_pool(name="psH", bufs=1, space="PSUM") as psH:
        h_acc = psH.tile([P, NBC * HID], F32, name="hacc")  # all 8 banks
        for hb, (k0, k1) in enumerate(head):
            xt_t = xtp.tile([P, k1 - k0, BSH], BF16, name="xtt")
            nc.sync.dma_start(xt_t, XT[:, k0:k1, :])
            for j2 in range((k1 - k0) // 2):
                w1 = w1p.tile([P, 2, HID], BF16, name="w1t")
                nc.sync.dma_start(w1, WE1[:, k0 + 2 * j2:k0 + 2 * j2 + 2, :])
                for jj in range(2):
                    for bc in range(NBC):
                        l1_mm(h_acc, xt_t, w1, k0 + j2 * 2 + jj, k0, jj, bc)
            if 2 <= hb < 2 + W2PRE:
                nc.sync.dma_start(w2c[:, hb - 2], t["wd2"][:, hb - 2, :, :])
        # tail: load all remaining tiles, then sweep bc-major
        xt_tail, w1_tail = [], []
        for k0, k1 in tail:
            xt_t = xtp.tile([P, k1 - k0, BSH], BF16, name="xtt")
            nc.sync.dma_start(xt_t, XT[:, k0:k1, :])
            xt_tail.append(xt_t)
            for j2 in range((k1 - k0) // 2):
                w1 = w1tl.tile([P, 2, HID], BF16, name="w1tt")
                nc.sync.dma_start(w1, WE1[:, k0 + 2 * j2:k0 + 2 * j2 + 2, :])
                w1_tail.append(w1)
        for bc in range(NBC):
            for ti, (k0, k1) in enumerate(tail):
                for j2 in range((k1 - k0) // 2):
                    for jj in range(2):
                        l1_mm(h_acc, xt_tail[ti], w1_tail[ti * 4 + j2],
                              k0 + j2 * 2 + jj, k0, jj, bc)
            relu_bc(h_acc, bc, htp)

    dT_cm = tc.tile_pool(name="dTp", bufs=1)
    dTp = dT_cm.__enter__()
    dT = dTp.tile([P, HKT, BSH], FP8, name="dT")  # 4 KB/partition

    # ---- Phase C: top-64 mask (bf16) + layer 2 -> dT (fp8) ----
    # ---- Phase D: out = sigmoid((dT.T @ Wd2_fp8) / 16), streaming Wd2 ----
    with tc.tile_pool(name="w2d", bufs=1) as w2dp, \
         tc.tile_pool(name="psO", bufs=6, space="PSUM") as psO, \
         tc.tile_pool(name="psT", bufs=2, space="PSUM") as psT, \
         tc.tile_pool(name="obp", bufs=8) as obp:
        w2d = w2dp.tile([P, NCH - W2PRE, HKT, 512], FP8, name="w2d")
        for i in range(NCH - W2PRE):
            nc.gpsimd.dma_start(w2d[:, i], t["wd2"][:, W2PRE + i, :, :])

        def phase_c(bc):
            """top-64 mask (DVE) -> hT -> layer 2 -> dT slice for one bc."""
            m8 = scr.tile([P, 8], BF16, name="m8")
            scratch = scr.tile([P, HID], BF16, name="scratch")
            src = h_sb[bc]
            for _ in range(TOPK // 8):
                nc.vector.max(out=m8, in_=src)
                nc.vector.match_replace(out=scratch, in_to_replace=m8,
                                        in_values=src, imm_value=0.0)
                src = scratch
            h_mask = hm.tile([P, HID], BF16, name="hmask")
            nc.vector.tensor_sub(h_mask, h_sb[bc], scratch)

            hT = htl.tile([P, HKT, P], BF16, name="hT")
            for g in range(2):
                ptc = psT.tile([P, 512], BF16, name="ptC")
                for j in range(4):
                    col = (g * 4 + j) * P
                    nc.tensor.transpose(ptc[:, j * P:(j + 1) * P],
                                        h_mask[:, col:col + P], ident)
                nc.scalar.copy(hT[:, g * 4:(g + 1) * 4, :],
                               ptc.rearrange("p (a b) -> p a b", a=4))

            d_bf = dbfp.tile([P, HID], BF16, name="dbf")
            for half in range(2):
                pd = psO.tile([P, 512], F32, name="pob")
                for k in range(HKT):
                    nc.tensor.matmul(pd, hT[:, k, :],
                                     wd1_sb[:, k, half * 512:(half + 1) * 512],
                                     start=(k == 0), stop=(k == HKT - 1))
                if has_bias:
                    d_tmp = dbfp.tile([P, 512], F32, name="dtmp")
                    nc.vector.tensor_add(d_tmp, pd, bd1_t[:, half * 512:(half + 1) * 512])
                    nc.scalar.activation(d_bf[:, half * 512:(half + 1) * 512], d_tmp, AF.Relu)
                else:
                    nc.scalar.activation(d_bf[:, half * 512:(half + 1) * 512], pd, AF.Relu)

            for g in range(2):
                ptd = psT.tile([P, 512], BF16, name="ptC")
                for j in range(4):
                    col = (g * 4 + j) * P
                    nc.tensor.transpose(ptd[:, j * P:(j + 1) * P],
                                        d_bf[:, col:col + P], ident)
                nc.scalar.copy(dT[:, g * 4:(g + 1) * 4, bc * P:(bc + 1) * P],
                               ptd.rearrange("p (a b) -> p a b", a=4))

        def phase_d(bcs):
            """layer 3 for the given bc chunks, streaming Wd2 (fp8 DoubleRow)."""
            for nch in range(NCH):
                ncol = nch * 512
                if has_bias:
                    bd2_sl = obp.tile([P, 512], F32, name="bd2sl")
                    nc.sync.dma_start(bd2_sl, bd2[None, ncol:ncol + 512]
                                      .to_broadcast([P, 512]))
                w2 = w2c[:, nch] if nch < W2PRE else w2d[:, nch - W2PRE]
                pos = {bc: psO.tile([P, 512], F32, name="pob") for bc in bcs}
                if D_JOUTER:
                    for j in range(HKT // 2):
                        for bc in bcs:
                            nc.tensor.matmul(pos[bc],
                                             dT[:, 2 * j:2 * j + 2, bc * P:(bc + 1) * P],
                                             w2[:, 2 * j:2 * j + 2, :],
                                             start=(j == 0), stop=(j == HKT // 2 - 1),
                                             perf_mode=mybir.MatmulPerfMode.DoubleRow)
                else:
                    for bc in bcs:
                        for j in range(HKT // 2):
                            nc.tensor.matmul(pos[bc],
                                             dT[:, 2 * j:2 * j + 2, bc * P:(bc + 1) * P],
                                             w2[:, 2 * j:2 * j + 2, :],
                                             start=(j == 0), stop=(j == HKT // 2 - 1),
                                             perf_mode=mybir.MatmulPerfMode.DoubleRow)
                for bc in bcs:
                    po = pos[bc]
                    o_sb = obp.tile([P, 512], BF16, name="osb")
                    if has_bias:
                        o_f32 = obp.tile([P, 512], F32, name="of32")
                        nc.vector.tensor_scalar_mul(o_f32, po, 1.0 / W2SCALE)
                        nc.vector.tensor_add(o_f32, o_f32, bd2_sl)
                        nc.scalar.activation(o_sb, o_f32, AF.Sigmoid)
                    else:
                        nc.scalar.activation(o_sb, po, AF.Sigmoid, scale=1.0 / W2SCALE)
                    nc.sync.dma_start(OUT[bc * P:(bc + 1) * P, ncol:ncol + 512], o_sb)

        if D_SPLIT:
            phase_c(0)
            phase_c(1)
            phase_d([0, 1])
            phase_c(2)
            phase_c(3)
            phase_d([2, 3])
        else:
            for bc in range(NBC):
                phase_c(bc)
            phase_d(list(range(NBC)))

    dT_cm.__exit__(None, None, None)
    for cm in reversed(pin_cms):
        cm.__exit__(None, None, None)
    w2c_cm.__exit__(None, None, None)
    hs_cm.__exit__(None, None, None)


def build_program(has_bias=False, rep=1, split_waits=True):
    nc = bass.Bass()
    t = {
        "xt": nc.dram_tensor("xt", [P, DKT, BSH], BF16, kind="ExternalInput")[:],
        "we1": nc.dram_tensor("we1", [P, DKT, HID], BF16, kind="ExternalInput")[:],
        "be1": nc.dram_tensor("be1", [HID], F32, kind="ExternalInput")[:],
        "wd1": nc.dram_tensor("wd1", [P, HKT, HID], BF16, kind="ExternalInput")[:],
        "bd1": nc.dram_tensor("bd1", [HID], F32, kind="ExternalInput")[:],
        "wd2": nc.dram_tensor("wd2", [P, NCH, HKT, 512], FP8, kind="ExternalInput")[:],
        "bd2": nc.dram_tensor("bd2", [D], F32, kind="ExternalInput")[:],
        "out": nc.dram_tensor("out", [BSH, D], BF16, kind="ExternalOutput")[:],
    }
    with TileContext(nc) as tc:
        _emit(nc, tc, t, has_bias, rep)
    if split_waits:
        _split_excess_waits(nc)
    return nc


_CACHE = {}


def _get_program(has_bias, rep=1):
    key = (has_bias, rep)
    if key not in _CACHE:
        _CACHE[key] = build_program(has_bias, rep)
    return _CACHE[key]


def _stage_weights(We1, Wd1, Wd2):
    """Host-side cast/retile of the replicated weights (shared by cores)."""
    bf16 = ml_dtypes.bfloat16
    f8 = ml_dtypes.float8_e4m3
    # we1[p, kt, n] = We1[kt*128 + p, n]
    we1_s = np.ascontiguousarray(
        We1.reshape(DKT, P, HID).transpose(1, 0, 2).astype(bf16))
    wd1_s = np.ascontiguousarray(
        Wd1.reshape(HKT, P, HID).transpose(1, 0, 2).astype(bf16))
    # wd2[p, nch, kt, n] = Wd2[kt*128 + p, nch*512 + n] * W2SCALE
    wd2_s = np.ascontiguousarray(
        (Wd2 * W2SCALE).reshape(HKT, P, NCH, 512).transpose(1, 2, 0, 3).astype(f8))
    return we1_s, wd1_s, wd2_s


def _stage_x(x2, c):
    """xt[p, kt, b] = x[c*BSH + b, kt*128 + p], bf16."""
    xs = x2[c * BSH:(c + 1) * BSH]                       # [512, 12288]
    xt = xs.T.reshape(DKT, P, BSH).transpose(1, 0, 2)    # [128, 96, 512]
    return np.ascontiguousarray(xt.astype(ml_dtypes.bfloat16))


def _make_in_maps(inputs):
    """Build per-core input maps from full-size f32 inputs dict."""
    X = np.asarray(inputs["X"], dtype=np.float32)
    We1 = np.asarray(inputs["We1"], dtype=np.float32)
    be1 = np.ascontiguousarray(np.asarray(inputs["be1"], dtype=np.float32))
    Wd1 = np.asarray(inputs["Wd1"], dtype=np.float32)
    bd1 = np.ascontiguousarray(np.asarray(inputs["bd1"], dtype=np.float32))
    Wd2 = np.asarray(inputs["Wd2"], dtype=np.float32)
    bd2 = np.ascontiguousarray(np.asarray(inputs["bd2"], dtype=np.float32))
    x2 = X.reshape(B, D)
    we1_s, wd1_s, wd2_s = _stage_weights(We1, Wd1, Wd2)
    in_maps = []
    for c in range(NCORES):
        in_maps.append({
            "xt": _stage_x(x2, c),
            "we1": we1_s, "be1": be1, "wd1": wd1_s, "bd1": bd1,
            "wd2": wd2_s, "bd2": bd2,
        })
    return in_maps


def kernel(X, We1, be1, Wd1, bd1, Wd2, bd2):
    X = np.asarray(X, dtype=np.float32)
    orig_shape = X.shape
    inputs = {"X": X, "We1": We1, "be1": be1, "Wd1": Wd1, "bd1": bd1,
              "Wd2": Wd2, "bd2": bd2}
    has_bias = bool(np.any(np.asarray(be1)) or np.any(np.asarray(bd1))
                    or np.any(np.asarray(bd2)))
    nc = _get_program(has_bias)
    in_maps = _make_in_maps(inputs)
    res = run_bass_kernel_spmd(nc, in_maps, list(range(NCORES)))
    out = np.concatenate([np.asarray(res.results[c]["out"]).astype(np.float32)
                          for c in range(NCORES)], axis=0)
    return out.reshape(orig_shape)


if __name__ == "__main__":
    rng = np.random.default_rng(0)
    Xs = rng.random((B, 3, 64, 64), dtype=np.float32)
    lim1 = np.float32(np.sqrt(6.0 / (D + HID)))
    limd = np.float32(np.sqrt(6.0 / (HID + HID)))
    lim2 = np.float32(np.sqrt(6.0 / (HID + D)))
    We1s = rng.uniform(-lim1, lim1, (D, HID)).astype(np.float32)
    Wd1s = rng.uniform(-limd, limd, (HID, HID)).astype(np.float32)
    Wd2s = rng.uniform(-lim2, lim2, (HID, D)).astype(np.float32)
    z1 = np.zeros(HID, np.float32)
    z2 = np.zeros(D, np.float32)
    got = kernel(Xs, We1s, z1, Wd1s, z1, Wd2s, z2)
    print("kernel output", got.shape, got.dtype, "mean", got.mean())



# revision 4
# speedup vs baseline: 1.1634x; 1.1634x over previous
"""Trainium2 Bass kernel for nn_DenseAE (dense autoencoder with top-k masking).

out = sigmoid(relu(ksparse(relu(x @ We1), 64) @ Wd1) @ Wd2)

Data-parallel over 8 NeuronCores: batch 4096 -> 512 rows/core, weights
replicated. All three matmul layers run in fp8e4 DoubleRow perf mode (2
k-tiles per matmul, 2x bf16 throughput): x and the weights are staged from
the host pre-transposed in fp8 (We1 scaled by 256, Wd1 by 128, Wd2 by 16;
descaled inside the relu/sigmoid activations). The top-64 mask runs on the
DVE in bf16 and is overlapped with the tensor engine via a bc-major layer-1
tail and per-bc-interleaved layer-3 emission. Output is written bf16 and
upcast on the host.
"""
import sys

sys.path.insert(0, "/opt/trn_rl_repo")

import numpy as np
import ml_dtypes

import concourse.bass as bass
import concourse.mybir as mybir
import concourse.tile as tile_mod
from concourse.bass_utils import run_bass_kernel_spmd
from concourse.masks import make_identity
from concourse.tile import TileContext
from bass_rust import ScopedClock

F32 = mybir.dt.float32
BF16 = mybir.dt.bfloat16
FP8 = mybir.dt.float8e4
AF = mybir.ActivationFunctionType
DR = mybir.MatmulPerfMode.DoubleRow

P = 128
D = 12288          # 3*64*64
HID = 1024
B = 4096
NCORES = 8
BSH = B // NCORES  # 512 rows per core
NBC = BSH // P     # 4 batch chunks of 128
DKT = D // P       # 96 contraction tiles for layer 1
NPAIR = DKT // 2   # 48 DoubleRow pairs for layer 1
HKT = HID // P     # 8 contraction tiles for layers 2/3
NCH = D // 512     # 24 output column chunks for layer 3
TOPK = 64
W1SCALE = 256.0    # We1 is staged as fp8(We1 * 256)
WD1SCALE = 128.0   # Wd1 is staged as fp8(Wd1 * 128)
W2SCALE = 16.0     # Wd2 is staged as fp8(Wd2 * 16)
TAILKB = 6         # last TAILKB k-blocks (8 kt each) of layer 1 run bc-major
                   # so h (and the DVE top-k) of early bc chunks starts well
                   # before layer 1 ends
W2PRE = 16         # Wd2 nch-chunks prefetched to SBUF during layer 1; the
                   # rest loads as layer-1 SBUF frees. All of Wd2 is read once
                   # and stays resident; every phase-D pass hits SBUF only.


def _patched_drain_and_barrier(self, tick_clock, wait_clock):
    # Tile's kernel-tail drain piles one wait per outstanding proc onto a
    # single Drain instruction; walrus CTRL_NO codegen has fewer wait slots.
    # Keep one wait on the drain and emit the rest as standalone waits.
    drain_inst = self.nc.sync.drain()
    wait_clock.add_sem_waits(drain_inst.ins, ScopedClock({None: tick_clock.global_clock}))
    si = drain_inst.ins.sync_info
    waits = list(si.on_wait or []) if si is not None else []
    if len(waits) > 1:
        si.on_wait = [waits[0]]
        sems = {h.name: h for h in self.sems.allocated().values()}
        for w in waits[1:]:
            assert w.wait_mode == "sem-ge-imm", w
            self.nc.sync.wait_ge(sems[w.ant_name], w.wait_value)
    self.nc.all_engine_barrier()
    assert self.sems is not None
    popped = self.nc._tile_sem_poison_stack.pop()
    assert popped is self._sem_poison
    self.nc.clear_and_free_semaphores(list(self.sems.allocated().values()))
    self.nc.all_engine_barrier()


tile_mod.TileContext._drain_and_barrier = _patched_drain_and_barrier

_WSPLIT_CTR = [0]


def _split_excess_waits(nc, cap=1):
    """This walrus build rejects instructions carrying more than ~1-2 sem
    waits ("Too many sync wait commands"), while Tile freely attaches
    several. Post-pass: keep `cap` waits on each instruction, move the rest
    onto fresh NOPs inserted immediately before it on the same engine."""
    for bb in nc.m.functions[0].blocks:
        lst = bb.instructions
        i = 0
        while i < len(lst):
            inst = lst[i]
            si = inst.sync_info
            waits = list(si.on_wait) if si is not None and si.on_wait else []
            if len(waits) > cap:
                si.on_wait = waits[-cap:]
                for w in waits[:-cap]:
                    nop = mybir.InstNoOp(name=f"wsplit-{_WSPLIT_CTR[0]}")
                    _WSPLIT_CTR[0] += 1
                    nop.engine = inst.engine
                    nop.sync_info = mybir.SyncInfo(on_wait=[w], on_update=[])
                    lst.insert(i, nop)
                    i += 1
            i += 1


def _emit(nc, tc, t, has_bias, rep=1):
    with tc.tile_pool(name="cpool", bufs=1) as const, \
         tc.tile_pool(name="wd1p", bufs=1) as wd1p:
        ident = const.tile([P, P], BF16, name="ident")
        make_identity(nc, ident)
        if has_bias:
            be1_t = const.tile([P, HID], F32, name="be1t")
            nc.sync.dma_start(be1_t, t["be1"][None, :].to_broadcast([P, HID]))
            bd1_t = const.tile([P, HID], F32, name="bd1t")
            nc.sync.dma_start(bd1_t, t["bd1"][None, :].to_broadcast([P, HID]))

        # Wd1 (1 MB fp8): load once up front, on the gpsimd queue so it does
        # not delay the first xt/we1 tiles on the sync queue.
        wd1_sb = wd1p.tile([P, HKT, HID], FP8, name="wd1sb")
        for kk in range(4):
            nc.gpsimd.dma_start(wd1_sb[:, 2 * kk:2 * kk + 2, :],
                                t["wd1"][:, 2 * kk:2 * kk + 2, :])

        for _ in range(rep):
            _emit_main(nc, tc, t, has_bias, ident, wd1_sb,
                       be1_t if has_bias else None, bd1_t if has_bias else None)


def _emit_main(nc, tc, t, has_bias, ident, wd1_sb, be1_t, bd1_t):
    XT, WE1, WD2, bd2, OUT = t["xt"], t["we1"], t["wd2"], t["bd2"], t["out"]

    hs_cm = tc.tile_pool(name="hsp", bufs=1)
    hsp = hs_cm.__enter__()
    h_sb = [hsp.tile([P, HID], BF16, name=f"hsb{bc}") for bc in range(NBC)]

    # Wd2 residency: first W2PRE chunks prefetch during layer 1 (interleaved
    # into the layer-1 sync-queue stream so they pace evenly), the rest
    # streams on gpsimd into space recycled from the layer-1 pools. All
    # phase-D passes then read SBUF only.
    w2c_cm = tc.tile_pool(name="w2c", bufs=1)
    w2cp = w2c_cm.__enter__()
    w2c = w2cp.tile([P, W2PRE, HKT, 512], FP8, name="w2c")

    # Pinned pools for the top-k working set: allocated alongside the
    # layer-1 pools so their first writes are not gated on layer-1 SBUF
    # frees (the DVE top-k must start during the layer-1 tail).
    pin_cms = [tc.tile_pool(name=n, bufs=2) for n in ("scr", "hm", "htl", "dbfp")]
    scr, hm, htl, dbfp = [cm.__enter__() for cm in pin_cms]

    # ---- Phase B: layer 1, h = relu(x @ We1), fp8 DoubleRow ----
    # Head runs kt-major (weights streamed once, small lead blocks shorten
    # time-to-first-matmul); the last TAILKB blocks run bc-major so h of
    # early bc chunks (and their DVE top-k) overlaps the tail matmuls.
    hk = DKT - 8 * TAILKB
    bounds = [b for b in [0, 2, 4, 8, 16, 24, 32, 40, 48] if b < hk] + [hk]
    head = list(zip(bounds[:-1], bounds[1:]))
    tail = [(hk + 8 * i, hk + 8 * (i + 1)) for i in range(TAILKB)]

    def l1_mm(h_acc, xt_t, w1, tp, t0, bc):
        first, last = tp == 0, tp == NPAIR - 1
        lhs = xt_t[:, 2 * (tp - t0):2 * (tp - t0) + 2, bc * P:(bc + 1) * P]
        nc.tensor.matmul(h_acc[:, 0:512], lhs,
                         w1[:, :, 0:512], start=first, stop=last, perf_mode=DR)
        nc.tensor.matmul(h_acc[:, 512:HID], lhs,
                         w1[:, :, 512:1024], start=first, stop=last, perf_mode=DR)

    def relu_bc(h_acc, bc, htp):
        if has_bias:
            h_tmp = htp.tile([P, HID], F32, name="htmp")
            nc.vector.tensor_scalar_mul(h_tmp, h_acc, 1.0 / W1SCALE)
            nc.vector.tensor_add(h_tmp, h_tmp, be1_t)
            nc.scalar.activation(h_sb[bc], h_tmp, AF.Relu)
        else:
            nc.scalar.activation(h_sb[bc], h_acc, AF.Relu, scale=1.0 / W1SCALE)

    with tc.tile_pool(name="xtp", bufs=3) as xtp, \
         tc.tile_pool(name="w1p", bufs=8) as w1p, \
         tc.tile_pool(name="xttl", bufs=TAILKB) as xttl, \
         tc.tile_pool(name="w1tl", bufs=4 * TAILKB) as w1tl, \
         tc.tile_pool(name="htmp", bufs=2) as htp, \
         tc.tile_pool(name="psH", bufs=1, space="PSUM") as psH:
        # one accumulator tile per bc chunk (2 banks each) so the relu of an
        # early bc is gated only on that bc's matmuls, not the whole layer
        h_accs = [psH.tile([P, HID], F32, name=f"hacc{bc}") for bc in range(NBC)]
        for hb, (k0, k1) in enumerate(head):
            xt_t = xtp.tile([P, k1 - k0, BSH], FP8, name="xtt")
            nc.scalar.dma_start(xt_t, XT[:, k0:k1, :])
            for j2 in range((k1 - k0) // 2):
                w1 = w1p.tile([P, 2, HID], FP8, name="w1t")
                nc.sync.dma_start(w1, WE1[:, k0 + 2 * j2:k0 + 2 * j2 + 2, :])
                for bc in range(NBC):
                    l1_mm(h_accs[bc], xt_t, w1, k0 // 2 + j2, k0 // 2, bc)
            if 2 <= hb and hb - 2 < W2PRE:
                nc.gpsimd.dma_start(w2c[:, hb - 2], t["wd2"][:, hb - 2, :, :])
        # tail: load all remaining tiles (w2 prefetch interleaved on the
        # gpsimd queue), then sweep bc-major
        xt_tail, w1_tail = [], []
        for ti, (k0, k1) in enumerate(tail):
            xt_t = xttl.tile([P, 8, BSH], FP8, name="xttt")
            nc.scalar.dma_start(xt_t, XT[:, k0:k1, :])
            xt_tail.append(xt_t)
            for j2 in range(4):
                w1 = w1tl.tile([P, 2, HID], FP8, name="w1tt")
                nc.sync.dma_start(w1, WE1[:, k0 + 2 * j2:k0 + 2 * j2 + 2, :])
                w1_tail.append(w1)
            pre = len(head) - 2 + ti
            if pre < W2PRE:
                nc.gpsimd.dma_start(w2c[:, pre], t["wd2"][:, pre, :, :])
        for pre in range(len(head) - 2 + TAILKB, W2PRE):
            nc.gpsimd.dma_start(w2c[:, pre], t["wd2"][:, pre, :, :])
        for bc in range(NBC):
            for ti, (k0, k1) in enumerate(tail):
                for j2 in range(4):
                    l1_mm(h_accs[bc], xt_tail[ti], w1_tail[ti * 4 + j2],
                          k0 // 2 + j2, k0 // 2, bc)
            relu_bc(h_accs[bc], bc, htp)

    dT_cm = tc.tile_pool(name="dTp", bufs=1)
    dTp = dT_cm.__enter__()
    dT = dTp.tile([P, HKT, BSH], FP8, name="dT")  # 4 KB/partition

    # ---- Phase C: top-64 mask (bf16, DVE) + layer 2 (fp8 DR) -> dT slice ----
    # ---- Phase D: out = sigmoid((dT.T @ Wd2_fp8) / 16), per bc chunk ----
    with tc.tile_pool(name="w2d", bufs=1) as w2dp, \
         tc.tile_pool(name="psO", bufs=6, space="PSUM") as psO, \
         tc.tile_pool(name="psT", bufs=2, space="PSUM") as psT, \
         tc.tile_pool(name="obp", bufs=8) as obp:
        w2d = w2dp.tile([P, NCH - W2PRE, HKT, 512], FP8, name="w2d")
        for i in range(NCH - W2PRE):
            nc.gpsimd.dma_start(w2d[:, i], t["wd2"][:, W2PRE + i, :, :])

        def phase_c(bc):
            """top-64 mask (DVE) -> hT (fp8) -> layer 2 -> dT slice for bc."""
            m8 = scr.tile([P, 8], BF16, name="m8")
            scratch = scr.tile([P, HID], BF16, name="scratch")
            src = h_sb[bc]
            for _ in range(TOPK // 8):
                nc.vector.max(out=m8, in_=src)
                nc.vector.match_replace(out=scratch, in_to_replace=m8,
                                        in_values=src, imm_value=0.0)
                src = scratch
            h_mask = hm.tile([P, HID], BF16, name="hmask")
            nc.vector.tensor_sub(h_mask, h_sb[bc], scratch)

            hT = htl.tile([P, HKT, P], FP8, name="hT")
            for g in range(2):
                ptc = psT.tile([P, 512], BF16, name="ptC")
                for j in range(4):
                    col = (g * 4 + j) * P
                    nc.tensor.transpose(ptc[:, j * P:(j + 1) * P],
                                        h_mask[:, col:col + P], ident)
                nc.scalar.copy(hT[:, g * 4:(g + 1) * 4, :],
                               ptc.rearrange("p (a b) -> p a b", a=4))

            d_bf = dbfp.tile([P, HID], BF16, name="dbf")
            for half in range(2):
                pd = psO.tile([P, 512], F32, name="pob")
                for j in range(HKT // 2):
                    nc.tensor.matmul(pd, hT[:, 2 * j:2 * j + 2, :],
                                     wd1_sb[:, 2 * j:2 * j + 2,
                                            half * 512:(half + 1) * 512],
                                     start=(j == 0), stop=(j == HKT // 2 - 1),
                                     perf_mode=DR)
                if has_bias:
                    d_tmp = dbfp.tile([P, 512], F32, name="dtmp")
                    nc.vector.tensor_scalar_mul(d_tmp, pd, 1.0 / WD1SCALE)
                    nc.vector.tensor_add(d_tmp, d_tmp,
                                         bd1_t[:, half * 512:(half + 1) * 512])
                    nc.scalar.activation(d_bf[:, half * 512:(half + 1) * 512],
                                         d_tmp, AF.Relu)
                else:
                    nc.scalar.activation(d_bf[:, half * 512:(half + 1) * 512],
                                         pd, AF.Relu, scale=1.0 / WD1SCALE)

            for g in range(2):
                ptd = psT.tile([P, 512], BF16, name="ptC")
                for j in range(4):
                    col = (g * 4 + j) * P
                    nc.tensor.transpose(ptd[:, j * P:(j + 1) * P],
                                        d_bf[:, col:col + P], ident)
                nc.scalar.copy(dT[:, g * 4:(g + 1) * 4, bc * P:(bc + 1) * P],
                               ptd.rearrange("p (a b) -> p a b", a=4))

        def phase_d(bc, nlo, nhi):
            """layer 3 for one bc chunk over nch in [nlo, nhi)."""
            for nch in range(nlo, nhi):
                ncol = nch * 512
                if has_bias:
                    bd2_sl = obp.tile([P, 512], F32, name="bd2sl")
                    nc.sync.dma_start(bd2_sl, bd2[None, ncol:ncol + 512]
                                      .to_broadcast([P, 512]))
                w2 = w2c[:, nch] if nch < W2PRE else w2d[:, nch - W2PRE]
                po = psO.tile([P, 512], F32, name="pob")
                for j in range(HKT // 2):
                    nc.tensor.matmul(po,
                                     dT[:, 2 * j:2 * j + 2, bc * P:(bc + 1) * P],
                                     w2[:, 2 * j:2 * j + 2, :],
                                     start=(j == 0), stop=(j == HKT // 2 - 1),
                                     perf_mode=DR)
                o_sb = obp.tile([P, 512], BF16, name="osb")
                if has_bias:
                    o_f32 = obp.tile([P, 512], F32, name="of32")
                    nc.vector.tensor_scalar_mul(o_f32, po, 1.0 / W2SCALE)
                    nc.vector.tensor_add(o_f32, o_f32, bd2_sl)
                    nc.scalar.activation(o_sb, o_f32, AF.Sigmoid)
                else:
                    nc.scalar.activation(o_sb, po, AF.Sigmoid, scale=1.0 / W2SCALE)
                nc.sync.dma_start(OUT[bc * P:(bc + 1) * P, ncol:ncol + 512], o_sb)

        # Emission order interleaves phase_c(bc+1) into phase_d of earlier
        # bc chunks so the tensor engine never waits on the serial DVE
        # top-k chain.
        phase_c(0)
        phase_d(0, 0, NCH // 2)
        phase_c(1)
        phase_d(0, NCH // 2, NCH)
        phase_c(2)
        phase_d(1, 0, NCH)
        phase_c(3)
        phase_d(2, 0, NCH)
        phase_d(3, 0, NCH)

    dT_cm.__exit__(None, None, None)
    for cm in reversed(pin_cms):
        cm.__exit__(None, None, None)
    w2c_cm.__exit__(None, None, None)
    hs_cm.__exit__(None, None, None)


def build_program(has_bias=False, rep=1, split_waits=True):
    nc = bass.Bass()
    t = {
        "xt": nc.dram_tensor("xt", [P, DKT, BSH], FP8, kind="ExternalInput")[:],
        "we1": nc.dram_tensor("we1", [P, DKT, HID], FP8, kind="ExternalInput")[:],
        "be1": nc.dram_tensor("be1", [HID], F32, kind="ExternalInput")[:],
        "wd1": nc.dram_tensor("wd1", [P, HKT, HID], FP8, kind="ExternalInput")[:],
        "bd1": nc.dram_tensor("bd1", [HID], F32, kind="ExternalInput")[:],
        "wd2": nc.dram_tensor("wd2", [P, NCH, HKT, 512], FP8, kind="ExternalInput")[:],
        "bd2": nc.dram_tensor("bd2", [D], F32, kind="ExternalInput")[:],
        "out": nc.dram_tensor("out", [BSH, D], BF16, kind="ExternalOutput")[:],
    }
    with TileContext(nc) as tc:
        _emit(nc, tc, t, has_bias, rep)
    if split_waits:
        _split_excess_waits(nc)
    return nc


_CACHE = {}


def _get_program(has_bias, rep=1):
    key = (has_bias, rep)
    if key not in _CACHE:
        _CACHE[key] = build_program(has_bias, rep)
    return _CACHE[key]


def _stage_weights(We1, Wd1, Wd2):
    """Host-side cast/retile of the replicated weights (shared by cores)."""
    f8 = ml_dtypes.float8_e4m3
    # we1[p, kt, n] = We1[kt*128 + p, n] * W1SCALE
    we1_s = np.ascontiguousarray(
        (We1 * W1SCALE).reshape(DKT, P, HID).transpose(1, 0, 2).astype(f8))
    wd1_s = np.ascontiguousarray(
        (Wd1 * WD1SCALE).reshape(HKT, P, HID).transpose(1, 0, 2).astype(f8))
    # wd2[p, nch, kt, n] = Wd2[kt*128 + p, nch*512 + n] * W2SCALE
    wd2_s = np.ascontiguousarray(
        (Wd2 * W2SCALE).reshape(HKT, P, NCH, 512).transpose(1, 2, 0, 3).astype(f8))
    return we1_s, wd1_s, wd2_s


def _stage_x(x2, c):
    """xt[p, kt, b] = x[c*BSH + b, kt*128 + p], fp8."""
    xs = x2[c * BSH:(c + 1) * BSH]                       # [512, 12288]
    xt = xs.T.reshape(DKT, P, BSH).transpose(1, 0, 2)    # [128, 96, 512]
    return np.ascontiguousarray(xt.astype(ml_dtypes.float8_e4m3))


def _make_in_maps(inputs):
    """Build per-core input maps from full-size f32 inputs dict."""
    X = np.asarray(inputs["X"], dtype=np.float32)
    We1 = np.asarray(inputs["We1"], dtype=np.float32)
    be1 = np.ascontiguousarray(np.asarray(inputs["be1"], dtype=np.float32))
    Wd1 = np.asarray(inputs["Wd1"], dtype=np.float32)
    bd1 = np.ascontiguousarray(np.asarray(inputs["bd1"], dtype=np.float32))
    Wd2 = np.asarray(inputs["Wd2"], dtype=np.float32)
    bd2 = np.ascontiguousarray(np.asarray(inputs["bd2"], dtype=np.float32))
    x2 = X.reshape(B, D)
    we1_s, wd1_s, wd2_s = _stage_weights(We1, Wd1, Wd2)
    in_maps = []
    for c in range(NCORES):
        in_maps.append({
            "xt": _stage_x(x2, c),
            "we1": we1_s, "be1": be1, "wd1": wd1_s, "bd1": bd1,
            "wd2": wd2_s, "bd2": bd2,
        })
    return in_maps


def kernel(X, We1, be1, Wd1, bd1, Wd2, bd2):
    X = np.asarray(X, dtype=np.float32)
    orig_shape = X.shape
    inputs = {"X": X, "We1": We1, "be1": be1, "Wd1": Wd1, "bd1": bd1,
              "Wd2": Wd2, "bd2": bd2}
    has_bias = bool(np.any(np.asarray(be1)) or np.any(np.asarray(bd1))
                    or np.any(np.asarray(bd2)))
    nc = _get_program(has_bias)
    in_maps = _make_in_maps(inputs)
    res = run_bass_kernel_spmd(nc, in_maps, list(range(NCORES)))
    out = np.concatenate([np.asarray(res.results[c]["out"]).astype(np.float32)
                          for c in range(NCORES)], axis=0)
    return out.reshape(orig_shape)


if __name__ == "__main__":
    rng = np.random.default_rng(0)
    Xs = rng.random((B, 3, 64, 64), dtype=np.float32)
    lim1 = np.float32(np.sqrt(6.0 / (D + HID)))
    limd = np.float32(np.sqrt(6.0 / (HID + HID)))
    lim2 = np.float32(np.sqrt(6.0 / (HID + D)))
    We1s = rng.uniform(-lim1, lim1, (D, HID)).astype(np.float32)
    Wd1s = rng.uniform(-limd, limd, (HID, HID)).astype(np.float32)
    Wd2s = rng.uniform(-lim2, lim2, (HID, D)).astype(np.float32)
    z1 = np.zeros(HID, np.float32)
    z2 = np.zeros(D, np.float32)
    got = kernel(Xs, We1s, z1, Wd1s, z1, Wd2s, z2)
    print("kernel output", got.shape, got.dtype, "mean", got.mean())


# revision 6
# speedup vs baseline: 1.3722x; 1.1794x over previous
"""Trainium2 Bass kernel for nn_DenseAE (dense autoencoder with top-k masking).

out = sigmoid(relu(ksparse(relu(x @ We1), 64) @ Wd1) @ Wd2)

Data-parallel over 8 NeuronCores: batch 4096 -> 512 rows/core, weights
replicated. All three matmul layers run in fp8e4 DoubleRow perf mode (2
k-tiles per matmul, 2x bf16 throughput): x and the weights are staged from
the host pre-transposed in fp8 (We1 scaled by 256, Wd1 by 128, Wd2 by 16;
descaled inside the relu/sigmoid activations). The top-64 mask runs on the
DVE in bf16 and is overlapped with the tensor engine via a bc-major layer-1
tail and per-bc-interleaved layer-3 emission. Output is written bf16 and
upcast on the host.
"""
import sys

sys.path.insert(0, "/opt/trn_rl_repo")

import numpy as np
import ml_dtypes

import concourse.bass as bass
import concourse.mybir as mybir
import concourse.tile as tile_mod
from concourse.bass_utils import run_bass_kernel_spmd
from concourse.masks import make_identity
from concourse.tile import TileContext
from bass_rust import ScopedClock

F32 = mybir.dt.float32
BF16 = mybir.dt.bfloat16
FP8 = mybir.dt.float8e4
AF = mybir.ActivationFunctionType
DR = mybir.MatmulPerfMode.DoubleRow

P = 128
D = 12288          # 3*64*64
HID = 1024
B = 4096
NCORES = 8
BSH = B // NCORES  # 512 rows per core
NBC = BSH // P     # 4 batch chunks of 128
DKT = D // P       # 96 contraction tiles for layer 1
NPAIR = DKT // 2   # 48 DoubleRow pairs for layer 1
HKT = HID // P     # 8 contraction tiles for layers 2/3
NCH = D // 512     # 24 output column chunks for layer 3
TOPK = 64
W1SCALE = 256.0    # We1 is staged as fp8(We1 * 256)
WD1SCALE = 128.0   # Wd1 is staged as fp8(Wd1 * 128)
W2SCALE = 16.0     # Wd2 is staged as fp8(Wd2 * 16)
TAILKB = 6         # last TAILKB k-blocks (8 kt each) of layer 1 run bc-major
                   # so h (and the DVE top-k) of early bc chunks starts well
                   # before layer 1 ends
W2PRE = 12         # Wd2 nch-chunks prefetched to SBUF during layer 1; the
                   # rest loads as layer-1 SBUF frees. All of Wd2 is read once
                   # and stays resident; every phase-D pass hits SBUF only.


def _patched_drain_and_barrier(self, tick_clock, wait_clock):
    # Tile's kernel-tail drain piles one wait per outstanding proc onto a
    # single Drain instruction; walrus CTRL_NO codegen has fewer wait slots.
    # Keep one wait on the drain and emit the rest as standalone waits.
    drain_inst = self.nc.sync.drain()
    wait_clock.add_sem_waits(drain_inst.ins, ScopedClock({None: tick_clock.global_clock}))
    si = drain_inst.ins.sync_info
    waits = list(si.on_wait or []) if si is not None else []
    if len(waits) > 1:
        si.on_wait = [waits[0]]
        sems = {h.name: h for h in self.sems.allocated().values()}
        for w in waits[1:]:
            assert w.wait_mode == "sem-ge-imm", w
            self.nc.sync.wait_ge(sems[w.ant_name], w.wait_value)
    self.nc.all_engine_barrier()
    assert self.sems is not None
    popped = self.nc._tile_sem_poison_stack.pop()
    assert popped is self._sem_poison
    self.nc.clear_and_free_semaphores(list(self.sems.allocated().values()))
    self.nc.all_engine_barrier()


tile_mod.TileContext._drain_and_barrier = _patched_drain_and_barrier

_WSPLIT_CTR = [0]


def _split_excess_waits(nc, cap=1):
    """This walrus build rejects instructions carrying more than ~1-2 sem
    waits ("Too many sync wait commands"), while Tile freely attaches
    several. Post-pass: keep `cap` waits on each instruction, move the rest
    onto fresh NOPs inserted immediately before it on the same engine."""
    for bb in nc.m.functions[0].blocks:
        lst = bb.instructions
        i = 0
        while i < len(lst):
            inst = lst[i]
            si = inst.sync_info
            waits = list(si.on_wait) if si is not None and si.on_wait else []
            if len(waits) > cap:
                si.on_wait = waits[-cap:]
                for w in waits[:-cap]:
                    nop = mybir.InstNoOp(name=f"wsplit-{_WSPLIT_CTR[0]}")
                    _WSPLIT_CTR[0] += 1
                    nop.engine = inst.engine
                    nop.sync_info = mybir.SyncInfo(on_wait=[w], on_update=[])
                    lst.insert(i, nop)
                    i += 1
            i += 1


def _emit(nc, tc, t, has_bias, rep=1):
    with tc.tile_pool(name="cpool", bufs=1) as const, \
         tc.tile_pool(name="wd1p", bufs=1) as wd1p:
        ident = const.tile([P, P], BF16, name="ident")
        make_identity(nc, ident)
        if has_bias:
            be1_t = const.tile([P, HID], F32, name="be1t")
            nc.sync.dma_start(be1_t, t["be1"][None, :].to_broadcast([P, HID]))
            bd1_t = const.tile([P, HID], F32, name="bd1t")
            nc.sync.dma_start(bd1_t, t["bd1"][None, :].to_broadcast([P, HID]))

        # Wd1 (1 MB fp8): load once up front, on the gpsimd queue so it does
        # not delay the first xt/we1 tiles on the sync queue.
        wd1_sb = wd1p.tile([P, HKT, HID], FP8, name="wd1sb")
        for kk in range(4):
            nc.gpsimd.dma_start(wd1_sb[:, 2 * kk:2 * kk + 2, :],
                                t["wd1"][:, 2 * kk:2 * kk + 2, :])

        for _ in range(rep):
            _emit_main(nc, tc, t, has_bias, ident, wd1_sb,
                       be1_t if has_bias else None, bd1_t if has_bias else None)


def _emit_main(nc, tc, t, has_bias, ident, wd1_sb, be1_t, bd1_t):
    XT, WE1, WD2, bd2, OUT = t["xt"], t["we1"], t["wd2"], t["bd2"], t["out"]

    hs_cm = tc.tile_pool(name="hsp", bufs=1)
    hsp = hs_cm.__enter__()
    h_sb = [hsp.tile([P, HID], BF16, name=f"hsb{bc}") for bc in range(NBC)]

    # Wd2 residency: first W2PRE chunks prefetch during layer 1 (interleaved
    # into the layer-1 sync-queue stream so they pace evenly), the rest
    # streams on gpsimd into space recycled from the layer-1 pools. All
    # phase-D passes then read SBUF only.
    w2c_cm = tc.tile_pool(name="w2c", bufs=1)
    w2cp = w2c_cm.__enter__()
    w2c = w2cp.tile([P, W2PRE, HKT, 512], FP8, name="w2c")

    # Pinned pools for the top-k working set: allocated alongside the
    # layer-1 pools so their first writes are not gated on layer-1 SBUF
    # frees (the DVE top-k must start during the layer-1 tail).
    pin_cms = [tc.tile_pool(name=n, bufs=2) for n in ("scr", "hm", "htl", "dbfp")]
    scr, hm, htl, dbfp = [cm.__enter__() for cm in pin_cms]

    # ---- Phase B: layer 1, h = relu(x @ We1), fp8 DoubleRow ----
    # Head runs kt-major (weights streamed once, small lead blocks shorten
    # time-to-first-matmul); the last TAILKB blocks run bc-major so h of
    # early bc chunks (and their DVE top-k) overlaps the tail matmuls.
    hk = DKT - 8 * TAILKB
    bounds = [b for b in [0, 2, 4, 8, 16, 24, 32, 40, 48] if b < hk] + [hk]
    head = list(zip(bounds[:-1], bounds[1:]))
    tail = [(hk + 8 * i, hk + 8 * (i + 1)) for i in range(TAILKB)]

    def l1_mm(h_acc, xt_t, w1, tp, t0, bc):
        first, last = tp == 0, tp == NPAIR - 1
        lhs = xt_t[:, 2 * (tp - t0):2 * (tp - t0) + 2, bc * P:(bc + 1) * P]
        nc.tensor.matmul(h_acc[:, 0:512], lhs,
                         w1[:, :, 0:512], start=first, stop=last, perf_mode=DR)
        nc.tensor.matmul(h_acc[:, 512:HID], lhs,
                         w1[:, :, 512:1024], start=first, stop=last, perf_mode=DR)

    def relu_bc(h_acc, bc, htp):
        if has_bias:
            h_tmp = htp.tile([P, HID], F32, name="htmp")
            nc.vector.tensor_scalar_mul(h_tmp, h_acc, 1.0 / W1SCALE)
            nc.vector.tensor_add(h_tmp, h_tmp, be1_t)
            nc.scalar.activation(h_sb[bc], h_tmp, AF.Relu)
        else:
            nc.scalar.activation(h_sb[bc], h_acc, AF.Relu, scale=1.0 / W1SCALE)

    with tc.tile_pool(name="xtp", bufs=3) as xtp, \
         tc.tile_pool(name="w1p", bufs=8) as w1p, \
         tc.tile_pool(name="xttl", bufs=TAILKB) as xttl, \
         tc.tile_pool(name="w1tl", bufs=4 * TAILKB) as w1tl, \
         tc.tile_pool(name="htmp", bufs=2) as htp, \
         tc.tile_pool(name="psH", bufs=1, space="PSUM") as psH:
        # one accumulator tile per bc chunk (2 banks each) so the relu of an
        # early bc is gated only on that bc's matmuls, not the whole layer
        h_accs = [psH.tile([P, HID], F32, name=f"hacc{bc}") for bc in range(NBC)]
        for hb, (k0, k1) in enumerate(head):
            xt_t = xtp.tile([P, k1 - k0, BSH], FP8, name="xtt")
            nc.scalar.dma_start(xt_t, XT[:, k0:k1, :])
            for j2 in range((k1 - k0) // 2):
                w1 = w1p.tile([P, 2, HID], FP8, name="w1t")
                nc.sync.dma_start(w1, WE1[:, k0 + 2 * j2:k0 + 2 * j2 + 2, :])
                for bc in range(NBC):
                    l1_mm(h_accs[bc], xt_t, w1, k0 // 2 + j2, k0 // 2, bc)
            if 2 <= hb and hb - 2 < W2PRE:
                nc.sync.dma_start(w2c[:, hb - 2], t["wd2"][:, hb - 2, :, :])
        # tail: load all remaining tiles (w2 prefetch interleaved on the
        # sync queue for pacing), then sweep bc-major
        xt_tail, w1_tail = [], []
        for ti, (k0, k1) in enumerate(tail):
            xt_t = xttl.tile([P, 8, BSH], FP8, name="xttt")
            nc.scalar.dma_start(xt_t, XT[:, k0:k1, :])
            xt_tail.append(xt_t)
            for j2 in range(4):
                w1 = w1tl.tile([P, 2, HID], FP8, name="w1tt")
                nc.sync.dma_start(w1, WE1[:, k0 + 2 * j2:k0 + 2 * j2 + 2, :])
                w1_tail.append(w1)
            pre = len(head) - 2 + ti
            if pre < W2PRE:
                nc.sync.dma_start(w2c[:, pre], t["wd2"][:, pre, :, :])
        for bc in range(NBC):
            for ti, (k0, k1) in enumerate(tail):
                for j2 in range(4):
                    l1_mm(h_accs[bc], xt_tail[ti], w1_tail[ti * 4 + j2],
                          k0 // 2 + j2, k0 // 2, bc)
            relu_bc(h_accs[bc], bc, htp)

    dT_cm = tc.tile_pool(name="dTp", bufs=1)
    dTp = dT_cm.__enter__()
    dT = dTp.tile([P, HKT, BSH], FP8, name="dT")  # 4 KB/partition

    # ---- Phase C: top-64 mask (bf16, DVE) + layer 2 (fp8 DR) -> dT slice ----
    # ---- Phase D: out = sigmoid((dT.T @ Wd2_fp8) / 16), per bc chunk ----
    with tc.tile_pool(name="w2d", bufs=1) as w2dp, \
         tc.tile_pool(name="psO", bufs=6, space="PSUM") as psO, \
         tc.tile_pool(name="psT", bufs=2, space="PSUM") as psT, \
         tc.tile_pool(name="obp", bufs=8) as obp:
        w2d = w2dp.tile([P, NCH - W2PRE, HKT, 512], FP8, name="w2d")
        for i in range(NCH - W2PRE):
            nc.gpsimd.dma_start(w2d[:, i], t["wd2"][:, W2PRE + i, :, :])

        def phase_c(bc):
            """top-64 mask (DVE) -> hT (fp8) -> layer 2 -> dT slice for bc."""
            m8 = scr.tile([P, 8], BF16, name="m8")
            scratch = scr.tile([P, HID], BF16, name="scratch")
            src = h_sb[bc]
            for _ in range(TOPK // 8):
                nc.vector.max(out=m8, in_=src)
                nc.vector.match_replace(out=scratch, in_to_replace=m8,
                                        in_values=src, imm_value=0.0)
                src = scratch
            h_mask = hm.tile([P, HID], BF16, name="hmask")
            nc.vector.tensor_sub(h_mask, h_sb[bc], scratch)

            hT = htl.tile([P, HKT, P], FP8, name="hT")
            for g in range(2):
                ptc = psT.tile([P, 512], BF16, name="ptC")
                for j in range(4):
                    col = (g * 4 + j) * P
                    nc.tensor.transpose(ptc[:, j * P:(j + 1) * P],
                                        h_mask[:, col:col + P], ident)
                nc.scalar.copy(hT[:, g * 4:(g + 1) * 4, :],
                               ptc.rearrange("p (a b) -> p a b", a=4))

            d_bf = dbfp.tile([P, HID], BF16, name="dbf")
            for half in range(2):
                pd = psO.tile([P, 512], F32, name="pob")
                for j in range(HKT // 2):
                    nc.tensor.matmul(pd, hT[:, 2 * j:2 * j + 2, :],
                                     wd1_sb[:, 2 * j:2 * j + 2,
                                            half * 512:(half + 1) * 512],
                                     start=(j == 0), stop=(j == HKT // 2 - 1),
                                     perf_mode=DR)
                if has_bias:
                    d_tmp = dbfp.tile([P, 512], F32, name="dtmp")
                    nc.vector.tensor_scalar_mul(d_tmp, pd, 1.0 / WD1SCALE)
                    nc.vector.tensor_add(d_tmp, d_tmp,
                                         bd1_t[:, half * 512:(half + 1) * 512])
                    nc.scalar.activation(d_bf[:, half * 512:(half + 1) * 512],
                                         d_tmp, AF.Relu)
                else:
                    nc.scalar.activation(d_bf[:, half * 512:(half + 1) * 512],
                                         pd, AF.Relu, scale=1.0 / WD1SCALE)

            for g in range(2):
                ptd = psT.tile([P, 512], BF16, name="ptC")
                for j in range(4):
                    col = (g * 4 + j) * P
                    nc.tensor.transpose(ptd[:, j * P:(j + 1) * P],
                                        d_bf[:, col:col + P], ident)
                nc.scalar.copy(dT[:, g * 4:(g + 1) * 4, bc * P:(bc + 1) * P],
                               ptd.rearrange("p (a b) -> p a b", a=4))

        def phase_d(bc, nlo, nhi):
            """layer 3 for one bc chunk over nch in [nlo, nhi)."""
            for nch in range(nlo, nhi):
                ncol = nch * 512
                if has_bias:
                    bd2_sl = obp.tile([P, 512], F32, name="bd2sl")
                    nc.sync.dma_start(bd2_sl, bd2[None, ncol:ncol + 512]
                                      .to_broadcast([P, 512]))
                w2 = w2c[:, nch] if nch < W2PRE else w2d[:, nch - W2PRE]
                po = psO.tile([P, 512], F32, name="pob")
                for j in range(HKT // 2):
                    nc.tensor.matmul(po,
                                     dT[:, 2 * j:2 * j + 2, bc * P:(bc + 1) * P],
                                     w2[:, 2 * j:2 * j + 2, :],
                                     start=(j == 0), stop=(j == HKT // 2 - 1),
                                     perf_mode=DR)
                o_sb = obp.tile([P, 512], BF16, name="osb")
                if has_bias:
                    o_f32 = obp.tile([P, 512], F32, name="of32")
                    nc.vector.tensor_scalar_mul(o_f32, po, 1.0 / W2SCALE)
                    nc.vector.tensor_add(o_f32, o_f32, bd2_sl)
                    nc.scalar.activation(o_sb, o_f32, AF.Sigmoid)
                else:
                    nc.scalar.activation(o_sb, po, AF.Sigmoid, scale=1.0 / W2SCALE)
                nc.sync.dma_start(OUT[bc * P:(bc + 1) * P, ncol:ncol + 512], o_sb)

        # Emission order interleaves phase_c(bc+1) into phase_d of earlier
        # bc chunks so the tensor engine never waits on the serial DVE
        # top-k chain.
        phase_c(0)
        phase_d(0, 0, NCH // 2)
        phase_c(1)
        phase_d(0, NCH // 2, NCH)
        phase_c(2)
        phase_d(1, 0, NCH)
        phase_c(3)
        phase_d(2, 0, NCH)
        phase_d(3, 0, NCH)

    dT_cm.__exit__(None, None, None)
    for cm in reversed(pin_cms):
        cm.__exit__(None, None, None)
    w2c_cm.__exit__(None, None, None)
    hs_cm.__exit__(None, None, None)


def build_program(has_bias=False, rep=1, split_waits=True):
    nc = bass.Bass()
    t = {
        "xt": nc.dram_tensor("xt", [P, DKT, BSH], FP8, kind="ExternalInput")[:],
        "we1": nc.dram_tensor("we1", [P, DKT, HID], FP8, kind="ExternalInput")[:],
        "be1": nc.dram_tensor("be1", [HID], F32, kind="ExternalInput")[:],
        "wd1": nc.dram_tensor("wd1", [P, HKT, HID], FP8, kind="ExternalInput")[:],
        "bd1": nc.dram_tensor("bd1", [HID], F32, kind="ExternalInput")[:],
        "wd2": nc.dram_tensor("wd2", [P, NCH, HKT, 512], FP8, kind="ExternalInput")[:],
        "bd2": nc.dram_tensor("bd2", [D], F32, kind="ExternalInput")[:],
        "out": nc.dram_tensor("out", [BSH, D], BF16, kind="ExternalOutput")[:],
    }
    with TileContext(nc) as tc:
        _emit(nc, tc, t, has_bias, rep)
    if split_waits:
        _split_excess_waits(nc)
    return nc


_CACHE = {}


def _get_program(has_bias, rep=1):
    key = (has_bias, rep)
    if key not in _CACHE:
        _CACHE[key] = build_program(has_bias, rep)
    return _CACHE[key]


def _stage_weights(We1, Wd1, Wd2):
    """Host-side cast/retile of the replicated weights (shared by cores)."""
    f8 = ml_dtypes.float8_e4m3
    # we1[p, kt, n] = We1[kt*128 + p, n] * W1SCALE
    we1_s = np.ascontiguousarray(
        (We1 * W1SCALE).reshape(DKT, P, HID).transpose(1, 0, 2).astype(f8))
    wd1_s = np.ascontiguousarray(
        (Wd1 * WD1SCALE).reshape(HKT, P, HID).transpose(1, 0, 2).astype(f8))
    # wd2[p, nch, kt, n] = Wd2[kt*128 + p, nch*512 + n] * W2SCALE
    wd2_s = np.ascontiguousarray(
        (Wd2 * W2SCALE).reshape(HKT, P, NCH, 512).transpose(1, 2, 0, 3).astype(f8))
    return we1_s, wd1_s, wd2_s


def _stage_x(x2, c):
    """xt[p, kt, b] = x[c*BSH + b, kt*128 + p], fp8."""
    xs = x2[c * BSH:(c + 1) * BSH]                       # [512, 12288]
    xt = xs.T.reshape(DKT, P, BSH).transpose(1, 0, 2)    # [128, 96, 512]
    return np.ascontiguousarray(xt.astype(ml_dtypes.float8_e4m3))


def _make_in_maps(inputs):
    """Build per-core input maps from full-size f32 inputs dict."""
    X = np.asarray(inputs["X"], dtype=np.float32)
    We1 = np.asarray(inputs["We1"], dtype=np.float32)
    be1 = np.ascontiguousarray(np.asarray(inputs["be1"], dtype=np.float32))
    Wd1 = np.asarray(inputs["Wd1"], dtype=np.float32)
    bd1 = np.ascontiguousarray(np.asarray(inputs["bd1"], dtype=np.float32))
    Wd2 = np.asarray(inputs["Wd2"], dtype=np.float32)
    bd2 = np.ascontiguousarray(np.asarray(inputs["bd2"], dtype=np.float32))
    x2 = X.reshape(B, D)
    we1_s, wd1_s, wd2_s = _stage_weights(We1, Wd1, Wd2)
    in_maps = []
    for c in range(NCORES):
        in_maps.append({
            "xt": _stage_x(x2, c),
            "we1": we1_s, "be1": be1, "wd1": wd1_s, "bd1": bd1,
            "wd2": wd2_s, "bd2": bd2,
        })
    return in_maps


def kernel(X, We1, be1, Wd1, bd1, Wd2, bd2):
    X = np.asarray(X, dtype=np.float32)
    orig_shape = X.shape
    inputs = {"X": X, "We1": We1, "be1": be1, "Wd1": Wd1, "bd1": bd1,
              "Wd2": Wd2, "bd2": bd2}
    has_bias = bool(np.any(np.asarray(be1)) or np.any(np.asarray(bd1))
                    or np.any(np.asarray(bd2)))
    nc = _get_program(has_bias)
    in_maps = _make_in_maps(inputs)
    res = run_bass_kernel_spmd(nc, in_maps, list(range(NCORES)))
    out = np.concatenate([np.asarray(res.results[c]["out"]).astype(np.float32)
                          for c in range(NCORES)], axis=0)
    return out.reshape(orig_shape)


if __name__ == "__main__":
    rng = np.random.default_rng(0)
    Xs = rng.random((B, 3, 64, 64), dtype=np.float32)
    lim1 = np.float32(np.sqrt(6.0 / (D + HID)))
    limd = np.float32(np.sqrt(6.0 / (HID + HID)))
    lim2 = np.float32(np.sqrt(6.0 / (HID + D)))
    We1s = rng.uniform(-lim1, lim1, (D, HID)).astype(np.float32)
    Wd1s = rng.uniform(-limd, limd, (HID, HID)).astype(np.float32)
    Wd2s = rng.uniform(-lim2, lim2, (HID, D)).astype(np.float32)
    z1 = np.zeros(HID, np.float32)
    z2 = np.zeros(D, np.float32)
    got = kernel(Xs, We1s, z1, Wd1s, z1, Wd2s, z2)
    print("kernel output", got.shape, got.dtype, "mean", got.mean())
